# revision 15
# baseline (speedup 1.0000x reference)
"""Trainium2 Bass kernel: multi-head cross-attention block (v2, all-bf16).

Reference computation (per batch b):
    q  = Wq @ x + bq            x = Vx[b] as (C, N=H*W)
    kv = Wkv @ Tx[b] + bkv      split per head h: rows 256h..256h+128 are k,
                                256h+128..256h+256 are v
    attn = softmax(q_h^T k_h * scale) over T
    o_h  = v_h @ attn^T
    out  = Wp @ concat_h(o_h) + bp

Sharding: pure data-parallel over B - 16 batches, 2 per NeuronCore.

v2 changes vs v1 (fp32r, 244us):
  * Everything bf16 (host-cast): halves DMA bytes and SBUF, enables the PE's
    fast-weight-load path; matmul row rate is identical to fp32r.  End-to-end
    max-rel-err ~3e-3 (vs 2e-2 gate), measured in a float64 numpy study.
  * One big DMA per tensor (weights laid out [128, KC*C] on host) instead of
    8: DMA-config sequencer time at startup drops ~6x.
  * Software-pipelined PE stream: attention matmuls of chunk i are
    interleaved with the q-projection of chunk i+1 (and the last chunk with
    the first out-projection), so the dependent attention matmuls
    (scores -> exp -> denom/out) never stall the PE - there is always an
    independent projection matmul between them.
  * Output written bf16 (host upcasts), split into 2 half-tile DMAs on
    alternating queues to cut the end-of-kernel DMA tail.

Softmax layout trick (kept from v1): scores are computed transposed
[t=77 part, n free], exp runs on that tile, and the softmax denominator is
broadcast across partitions by a ones[77,128]^T @ E matmul.  No
max-subtraction: |scores*scale| <= ~5 for this data scale.
"""

import numpy as np

NCORES = 8
B, C, N, T = 16, 1024, 1024, 77
NH, HD = 8, 128
BPC = B // NCORES        # batches per core
TB = 80                  # batch-1 column offset in packed-T tiles (16B-aligned)
T2 = TB + T              # used packed-T width (b0 at 0, b1 at TB)
TP = 160                 # padded packed-T width
NCHUNK = 512             # n-tile (free dim) size
NCH = N // NCHUNK        # chunks per batch
NPAIR = BPC * NCH        # (batch, chunk) pairs per core
KC = C // 128            # contraction tiles
SCALE = float(HD) ** -0.5
Q_FP8 = False            # q-projection via fp8e4 DoubleRow (2x PE rate)
QSW = 512.0              # fp8 weight scale (Wq*QSW quantized)
QSX = 16.0               # fp8 activation scale
NG = 4                   # DoubleRow groups (256-contraction each)

_CACHE = {}


def _build_module():
    from contextlib import ExitStack

    import concourse.bacc as bacc
    import concourse.mybir as mybir
    import concourse.tile as tile

    f32 = mybir.dt.float32
    bf16 = mybir.dt.bfloat16
    Id = mybir.ActivationFunctionType.Identity
    Exp = mybir.ActivationFunctionType.Exp

    nc = bacc.Bacc("TRN2", debug=False, enable_asserts=False,
                   num_devices=NCORES)

    tx = nc.dram_tensor("tx", [128, KC * TP], bf16, kind="ExternalInput").ap()
    wkvk = nc.dram_tensor("wkvk", [128, KC * C], bf16,
                          kind="ExternalInput").ap()
    wkvv = nc.dram_tensor("wkvv", [128, KC * C], bf16,
                          kind="ExternalInput").ap()
    fp8 = mybir.dt.float8e4
    xdt = fp8 if Q_FP8 else bf16
    wq = nc.dram_tensor("wq", [128, KC * C], xdt, kind="ExternalInput").ap()
    wp = nc.dram_tensor("wp", [128, KC * C], bf16, kind="ExternalInput").ap()
    xd = nc.dram_tensor("x", [NPAIR, 128, KC * NCHUNK], xdt,
                        kind="ExternalInput").ap()
    cf = nc.dram_tensor("cf32", [128, 3 * KC], f32, kind="ExternalInput").ap()
    onesd = nc.dram_tensor("onesd", [128, TP], bf16,
                           kind="ExternalInput").ap()
    bvr = nc.dram_tensor("bvr", [1, C], bf16, kind="ExternalInput").ap()
    out = nc.dram_tensor("out", [NPAIR, 128, KC * NCHUNK], bf16,
                         kind="ExternalOutput").ap()

    with tile.TileContext(nc) as tc, ExitStack() as ctx:
        import concourse.bass_isa as bass_isa
        c_p = ctx.enter_context(tc.tile_pool(name="c", bufs=1))
        w_p = ctx.enter_context(tc.tile_pool(name="w", bufs=1))
        kvw_p = ctx.enter_context(tc.tile_pool(name="kvw", bufs=1))
        tx_p = ctx.enter_context(tc.tile_pool(name="txp", bufs=1))
        x_p = ctx.enter_context(tc.tile_pool(name="x", bufs=1))
        q_p = ctx.enter_context(tc.tile_pool(name="q", bufs=2))
        k_p = ctx.enter_context(tc.tile_pool(name="k", bufs=1))
        vt_p = ctx.enter_context(tc.tile_pool(name="vt", bufs=1))
        e_p = ctx.enter_context(tc.tile_pool(name="e", bufs=4))
        ri_p = ctx.enter_context(tc.tile_pool(name="ri", bufs=3))
        dn_p = ctx.enter_context(tc.tile_pool(name="dn", bufs=3))
        on_p = ctx.enter_context(tc.tile_pool(name="on", bufs=1))
        os_p = ctx.enter_context(tc.tile_pool(name="os", bufs=4))
        ps = ctx.enter_context(tc.tile_pool(name="ps", bufs=8, space="PSUM"))

        # ---- DMA configs ----------------------------------------------
        # DMA engines are shared round-robin across active queues, so all
        # inputs go on ONE queue (sync) in exact first-need order - the
        # critical prologue transfers (tx+wkvk for k-proj) then get the
        # full aggregate DMA bandwidth instead of 1/3 of it.
        H = KC * C // 2
        Q4 = KC * C // 4
        XH = KC * NCHUNK // 2
        TXH = KC * TP // 2
        txs = tx_p.tile([128, KC * TP], bf16, name="txs", tag="tx")
        nc.sync.dma_start(txs[:, 0:TXH], tx[:, 0:TXH])
        nc.sync.dma_start(txs[:, TXH:], tx[:, TXH:])
        wkvk_sb = kvw_p.tile([128, KC * C], bf16, name="wkvk_sb", tag="wk")
        for j in range(4):
            nc.sync.dma_start(wkvk_sb[:, j * Q4:(j + 1) * Q4],
                              wkvk[:, j * Q4:(j + 1) * Q4])
        cf_sb = c_p.tile([128, 3 * KC], f32, name="cf_sb", tag="cf")
        nc.sync.dma_start(cf_sb, cf)
        ones_sb = c_p.tile([128, TP], bf16, name="ones_sb", tag="o1")
        nc.sync.dma_start(ones_sb, onesd)
        qdt = fp8 if Q_FP8 else bf16
        xs = [x_p.tile([128, KC * NCHUNK], qdt, name=f"x{i}", tag=f"x{i}")
              for i in range(NPAIR)]
        wq_sb = w_p.tile([128, KC * C], qdt, name="wq_sb", tag="wq")
        XQ = KC * NCHUNK // 4
        for j in range(4):
            nc.sync.dma_start(xs[0][:, j * XQ:(j + 1) * XQ],
                              xd[0, :, j * XQ:(j + 1) * XQ])
            nc.sync.dma_start(wq_sb[:, j * Q4:(j + 1) * Q4],
                              wq[:, j * Q4:(j + 1) * Q4])
        if Q_FP8:
            # DoubleRow views: [p, group, plane(2), inner]
            xv = [xt[:, :].rearrange("p (g two n) -> p g two n",
                                     g=NG, two=2) for xt in xs]
            wqv = wq_sb[:, :].rearrange("p (g two d) -> p g two d",
                                        g=NG, two=2)
        wkvv_sb = kvw_p.tile([128, KC * C], bf16, name="wkvv_sb", tag="wv")
        nc.sync.dma_start(wkvv_sb[:, 0:H], wkvv[:, 0:H])
        nc.sync.dma_start(wkvv_sb[:, H:], wkvv[:, H:])
        bv_sb = c_p.tile([1, C], bf16, name="bv_sb", tag="bv")
        nc.sync.dma_start(bv_sb, bvr)
        nc.sync.dma_start(xs[1][:, 0:XH], xd[1, :, 0:XH])
        nc.sync.dma_start(xs[1][:, XH:], xd[1, :, XH:])
        wp_sb = w_p.tile([128, KC * C], bf16, name="wp_sb", tag="wp")
        nc.sync.dma_start(wp_sb[:, 0:H], wp[:, 0:H])
        nc.sync.dma_start(wp_sb[:, H:], wp[:, H:])
        for i in (2, 3):
            nc.sync.dma_start(xs[i][:, 0:XH], xd[i, :, 0:XH])
            nc.sync.dma_start(xs[i][:, XH:], xd[i, :, XH:])

        bq_c = cf_sb[:, 0:KC]
        bp_c = cf_sb[:, KC:2 * KC]
        bk_c = cf_sb[:, 2 * KC:3 * KC]

        # ---- k-proj: both batches packed along T ----------------------
        kps = [ps.tile([128, TP], f32, name=f"kps{h}", tag="ps")
               for h in range(NH)]
        for cc in range(KC):
            rhs = txs[:, cc * TP:(cc + 1) * TP]
            for h in range(NH):
                lhs = wkvk_sb[:, cc * C + h * HD:cc * C + (h + 1) * HD]
                nc.tensor.matmul(kps[h], lhs, rhs,
                                 start=(cc == 0), stop=(cc == KC - 1))
        k_sb = k_p.tile([128, NH * TP], bf16, name="k_sb", tag="k")
        for h in range(NH):
            nc.scalar.activation(k_sb[:, h * TP:(h + 1) * TP], kps[h], Id,
                                 bias=bk_c[:, h:h + 1])

        # ---- q-proj (fp8 DoubleRow when Q_FP8, else bf16) -------------
        DR = mybir.MatmulPerfMode.DoubleRow
        QDS = 1.0 / (QSW * QSX) if Q_FP8 else 1.0

        def qp_mms(qps, i, d, g0, g1):
            """Contraction-group matmuls g0..g1-1 for d-tile d of pair i."""
            if Q_FP8:
                for g in range(g0, g1):
                    nc.tensor.matmul(qps, wqv[:, g, :, d * HD:(d + 1) * HD],
                                     xv[i][:, g], start=(g == 0),
                                     stop=(g == NG - 1), perf_mode=DR)
            else:
                for cc in range(2 * g0, 2 * g1):
                    lhs = wq_sb[:, cc * C + d * HD:cc * C + (d + 1) * HD]
                    nc.tensor.matmul(qps, lhs,
                                     xs[i][:, cc * NCHUNK:(cc + 1) * NCHUNK],
                                     start=(cc == 0), stop=(cc == KC - 1))

        def qproj(i, interleave=None):
            """Emit q-proj for pair i. interleave=None -> group-outer (8
            psums, paced by DMA); else returns 8 per-d-tile emit closures."""
            q_sb = q_p.tile([128, KC * NCHUNK], bf16, name=f"q{i}", tag="q")

            def evac(d, qps):
                nc.scalar.activation(
                    q_sb[:, d * NCHUNK:(d + 1) * NCHUNK], qps, Id,
                    scale=QDS, bias=bq_c[:, d:d + 1])

            def emit_d(d):
                qps = ps.tile([128, NCHUNK], f32, name=f"qps{i}_{d}",
                              tag="ps")
                qp_mms(qps, i, d, 0, NG)
                evac(d, qps)

            if interleave is None:
                qps_l = [ps.tile([128, NCHUNK], f32, name=f"qps{i}_{d}",
                                 tag="ps") for d in range(KC)]
                for g in range(NG):
                    for d in range(KC):
                        qp_mms(qps_l[d], i, d, g, g + 1)
                for d in range(KC):
                    evac(d, qps_l[d])
                return q_sb
            return q_sb, [lambda d=d: emit_d(d) for d in range(KC)]

        q_t = [None] * NPAIR
        q_t[0] = qproj(0)

        # ---- v-proj: vt[b] = (Tx[b]^T Wv^T + bv) as [77, C] -----------
        vt_sb = []
        for b in range(BPC):
            vt = vt_p.tile([T, C], bf16, name=f"vt{b}", tag=f"vt{b}")
            for half in range(2):
                vps = ps.tile([T, NCHUNK], f32, name=f"vps{b}_{half}",
                              tag="ps")
                for cc in range(KC):
                    lhsT = txs[:, cc * TP + b * TB:cc * TP + b * TB + T]
                    rhs = wkvv_sb[:, cc * C + half * NCHUNK:
                                  cc * C + (half + 1) * NCHUNK]
                    nc.tensor.matmul(vps, lhsT, rhs,
                                     start=(cc == 0), stop=False)
                nc.tensor.matmul(vps, ones_sb[0:1, 0:T],
                                 bv_sb[:, half * NCHUNK:(half + 1) * NCHUNK],
                                 start=False, stop=True)
                nc.scalar.copy(vt[:, half * NCHUNK:(half + 1) * NCHUNK], vps)
            vt_sb.append(vt)

        # ---- chunk loop: A(i) interleaved with partner stream ---------
        on_t = [[None] * NH for _ in range(NPAIR)]

        def attention(i, partner):
            """Emit attention for pair i, interleaving partner closures."""
            b = i // NCH
            q_sb = q_t[i]
            sps_l = {}
            e_l = {}

            def sc(h):
                sps = ps.tile([T, NCHUNK], f32, name=f"sps{i}_{h}", tag="ps")
                nc.tensor.matmul(sps, k_sb[:, h * TP + b * TB:
                                           h * TP + b * TB + T],
                                 q_sb[:, h * NCHUNK:(h + 1) * NCHUNK])
                e_sb = e_p.tile([T, NCHUNK], bf16, name=f"e{i}_{h}", tag="e")
                nc.scalar.activation(e_sb, sps, Exp, scale=SCALE)
                e_l[h] = e_sb

            def dn_out(h):
                # softmax denominator: cross-partition sum of E on gpsimd
                # (saves a 512-row PE matmul per head)
                d77 = dn_p.tile([T, NCHUNK], f32, name=f"d77_{i}_{h}",
                                tag="d77")
                nc.gpsimd.partition_all_reduce(d77, e_l[h], channels=T,
                                               reduce_op=bass_isa.ReduceOp.add)
                den = dn_p.tile([128, NCHUNK], f32, name=f"dn{i}_{h}",
                                tag="dn")
                nc.gpsimd.partition_broadcast(den, d77[0:1, :])
                ri = ri_p.tile([128, NCHUNK], f32, name=f"ri{i}_{h}",
                               tag="ri")
                nc.vector.reciprocal_approx_fast(ri, den)
                ops = ps.tile([128, NCHUNK], f32, name=f"ops{i}_{h}",
                              tag="ps")
                nc.tensor.matmul(ops, vt_sb[b][:, h * HD:(h + 1) * HD],
                                 e_l[h])
                on = on_p.tile([128, NCHUNK], bf16, name=f"on{i}_{h}",
                               tag=f"on{i}_{h}")
                nc.vector.tensor_mul(on, ops, ri)
                on_t[i][h] = on

            p = list(partner)
            sc(0)
            sc(1)
            if p:
                p.pop(0)()
            sc(2)
            for h in range(NH):
                dn_out(h)
                if h + 3 < NH:
                    sc(h + 3)
                if p:
                    p.pop(0)()
            while p:
                p.pop(0)()

        def pproj(i, interleave=False):
            """Emit p-proj for pair i; returns 8 closures if interleave."""
            def emit_e(e):
                fps = ps.tile([128, NCHUNK], f32, name=f"fps{i}_{e}",
                              tag="ps")
                for d in range(KC):
                    lhs = wp_sb[:, d * C + e * HD:d * C + (e + 1) * HD]
                    nc.tensor.matmul(fps, lhs, on_t[i][d],
                                     start=(d == 0), stop=(d == KC - 1))
                osb = os_p.tile([128, NCHUNK], bf16, name=f"os{i}_{e}",
                                tag="os")
                nc.scalar.activation(osb, fps, Id, bias=bp_c[:, e:e + 1])
                hm = NCHUNK // 2
                o0 = e * NCHUNK
                nc.sync.dma_start(out[i, :, o0:o0 + hm], osb[:, 0:hm])
                nc.sync.dma_start(out[i, :, o0 + hm:o0 + NCHUNK],
                                  osb[:, hm:])

            cl = [lambda e=e: emit_e(e) for e in range(KC)]
            if interleave:
                return cl
            for c in cl:
                c()

        # A0+qp1, A1+qp2, A2+qp3, A3+pp0, pp1, pp2, pp3
        for i in range(NPAIR):
            if i + 1 < NPAIR:
                q_t[i + 1], partner = qproj(i + 1, interleave=True)
            else:
                partner = pproj(0, interleave=True)
            attention(i, partner)
        for i in range(1, NPAIR):
            pproj(i)

    nc.compile()
    return nc


def _host_prep(Vx, Tx, Wq, bq, Wkv, bkv, Wp, bp):
    import ml_dtypes
    bf = ml_dtypes.bfloat16
    f = np.float32

    def wtile(w_t):
        # [C(row c), C(col d)] -> [128, KC*C] with block cc at cols cc*C
        return np.ascontiguousarray(
            w_t.reshape(KC, 128, C).transpose(1, 0, 2).reshape(128, KC * C)
        ).astype(bf)

    def q8(v, s):
        return np.clip(np.asarray(v, f) * s, -240.0,
                       240.0).astype(ml_dtypes.float8_e4m3)

    Wq = np.asarray(Wq, f)
    Wkv4 = np.asarray(Wkv, f).reshape(NH, 2, HD, C)
    Wp = np.asarray(Wp, f)
    if Q_FP8:
        # DoubleRow layout: flat col = g*2*C + plane*C + d,
        # value = Wq.T[g*256 + plane*128 + p, d] * QSW in fp8e4
        wq_h = np.ascontiguousarray(
            q8(Wq.T, QSW).reshape(NG, 2, 128, C)
            .transpose(2, 0, 1, 3).reshape(128, KC * C))
    else:
        wq_h = wtile(Wq.T)
    wkvk_h = wtile(Wkv4[:, 0].reshape(C, C).T)
    wkvv_h = wtile(Wkv4[:, 1].reshape(C, C).T)
    wp_h = wtile(Wp.T)

    cf32 = np.zeros((128, 3 * KC), f)
    cf32[:, 0:KC] = np.asarray(bq, f).reshape(KC, 128).T
    cf32[:, KC:2 * KC] = np.asarray(bp, f).reshape(KC, 128).T
    bkv3 = np.asarray(bkv, f).reshape(NH, 2, HD)
    cf32[:, 2 * KC:3 * KC] = bkv3[:, 0].T
    bvr_h = np.ascontiguousarray(bkv3[:, 1].reshape(1, C)).astype(bf)
    ones_h = np.ones((128, TP), bf)

    Vx3 = np.asarray(Vx, f).reshape(B, C, N)
    TxA = np.asarray(Tx, f)

    shared = {"wq": wq_h, "wkvk": wkvk_h, "wkvv": wkvv_h, "wp": wp_h,
              "cf32": cf32, "onesd": ones_h, "bvr": bvr_h}
    in_maps = []
    for i in range(NCORES):
        m = dict(shared)
        xb = Vx3[i * BPC:(i + 1) * BPC]
        if Q_FP8:
            m["x"] = np.ascontiguousarray(
                q8(xb, QSX).reshape(BPC, NG, 2, 128, NCH, NCHUNK)
                .transpose(0, 4, 3, 1, 2, 5)
                .reshape(NPAIR, 128, KC * NCHUNK))
        else:
            m["x"] = np.ascontiguousarray(
                xb.reshape(BPC, KC, 128, NCH, NCHUNK)
                .transpose(0, 3, 2, 1, 4).reshape(NPAIR, 128, KC * NCHUNK)
            ).astype(bf)
        txh = np.zeros((128, KC * TP), f)
        for cc in range(KC):
            for b2 in range(BPC):
                txh[:, cc * TP + b2 * TB:cc * TP + b2 * TB + T] = \
                    TxA[i * BPC + b2, cc * 128:(cc + 1) * 128, :]
        m["tx"] = txh.astype(bf)
        in_maps.append(m)
    return in_maps


def _unshard_core(arr):
    """[NPAIR, 128, KC*NCHUNK] bf16 -> [BPC, C, N] float32."""
    a = np.asarray(arr).astype(np.float32)
    return (a.reshape(BPC, NCH, 128, KC, NCHUNK)
            .transpose(0, 3, 2, 1, 4).reshape(BPC, C, N))


def get_module():
    if "nc" not in _CACHE:
        _CACHE["nc"] = _build_module()
    return _CACHE["nc"]


def kernel(**inputs):
    from concourse.bass_utils import run_bass_kernel_spmd

    nc = get_module()
    in_maps = _host_prep(**inputs)
    res = run_bass_kernel_spmd(nc, in_maps, core_ids=list(range(NCORES)))
    outs = [_unshard_core(res.results[i]["out"]) for i in range(NCORES)]
    full = np.concatenate(outs, axis=0).reshape(B, C, 32, 32)
    return np.ascontiguousarray(full.astype(np.float32))


# revision 17
# speedup vs baseline: 1.3893x; 1.3893x over previous
"""Trainium2 Bass kernel: multi-head cross-attention block (v2, all-bf16).

Reference computation (per batch b):
    q  = Wq @ x + bq            x = Vx[b] as (C, N=H*W)
    kv = Wkv @ Tx[b] + bkv      split per head h: rows 256h..256h+128 are k,
                                256h+128..256h+256 are v
    attn = softmax(q_h^T k_h * scale) over T
    o_h  = v_h @ attn^T
    out  = Wp @ concat_h(o_h) + bp

Sharding: pure data-parallel over B - 16 batches, 2 per NeuronCore.

v2 changes vs v1 (fp32r, 244us):
  * Everything bf16 (host-cast): halves DMA bytes and SBUF, enables the PE's
    fast-weight-load path; matmul row rate is identical to fp32r.  End-to-end
    max-rel-err ~3e-3 (vs 2e-2 gate), measured in a float64 numpy study.
  * One big DMA per tensor (weights laid out [128, KC*C] on host) instead of
    8: DMA-config sequencer time at startup drops ~6x.
  * Software-pipelined PE stream: attention matmuls of chunk i are
    interleaved with the q-projection of chunk i+1 (and the last chunk with
    the first out-projection), so the dependent attention matmuls
    (scores -> exp -> denom/out) never stall the PE - there is always an
    independent projection matmul between them.
  * Output written bf16 (host upcasts), split into 2 half-tile DMAs on
    alternating queues to cut the end-of-kernel DMA tail.

Softmax layout trick (kept from v1): scores are computed transposed
[t=77 part, n free], exp runs on that tile, and the softmax denominator is
broadcast across partitions by a ones[77,128]^T @ E matmul.  No
max-subtraction: |scores*scale| <= ~5 for this data scale.
"""

import numpy as np

NCORES = 8
B, C, N, T = 16, 1024, 1024, 77
NH, HD = 8, 128
BPC = B // NCORES        # batches per core
TB = 80                  # batch-1 column offset in packed-T tiles (16B-aligned)
T2 = TB + T              # used packed-T width (b0 at 0, b1 at TB)
TP = 160                 # padded packed-T width
NCHUNK = 512             # n-tile (free dim) size
NCH = N // NCHUNK        # chunks per batch
NPAIR = BPC * NCH        # (batch, chunk) pairs per core
KC = C // 128            # contraction tiles
SCALE = float(HD) ** -0.5
Q_FP8 = False            # q-projection via fp8e4 DoubleRow (2x PE rate)
QSW = 512.0              # fp8 weight scale (Wq*QSW quantized)
QSX = 16.0               # fp8 activation scale
NG = 4                   # DoubleRow groups (256-contraction each)

_CACHE = {}


def _build_module():
    from contextlib import ExitStack

    import concourse.bacc as bacc
    import concourse.mybir as mybir
    import concourse.tile as tile

    f32 = mybir.dt.float32
    bf16 = mybir.dt.bfloat16
    Id = mybir.ActivationFunctionType.Identity
    Exp = mybir.ActivationFunctionType.Exp

    nc = bacc.Bacc("TRN2", debug=False, enable_asserts=False,
                   num_devices=NCORES)

    tx = nc.dram_tensor("tx", [128, KC * TP], bf16, kind="ExternalInput").ap()
    wkvk = nc.dram_tensor("wkvk", [128, KC * C], bf16,
                          kind="ExternalInput").ap()
    wkvv = nc.dram_tensor("wkvv", [128, KC * C], bf16,
                          kind="ExternalInput").ap()
    fp8 = mybir.dt.float8e4
    xdt = fp8 if Q_FP8 else bf16
    wq = nc.dram_tensor("wq", [128, KC * C], xdt, kind="ExternalInput").ap()
    wp = nc.dram_tensor("wp", [128, KC * C], bf16, kind="ExternalInput").ap()
    xd = nc.dram_tensor("x", [NPAIR, 128, KC * NCHUNK], xdt,
                        kind="ExternalInput").ap()
    cf = nc.dram_tensor("cf32", [128, 3 * KC], f32, kind="ExternalInput").ap()
    onesd = nc.dram_tensor("onesd", [128, TP], bf16,
                           kind="ExternalInput").ap()
    bvr = nc.dram_tensor("bvr", [1, C], bf16, kind="ExternalInput").ap()
    out = nc.dram_tensor("out", [NPAIR, 128, KC * NCHUNK], bf16,
                         kind="ExternalOutput").ap()

    with tile.TileContext(nc) as tc, ExitStack() as ctx:
        c_p = ctx.enter_context(tc.tile_pool(name="c", bufs=1))
        w_p = ctx.enter_context(tc.tile_pool(name="w", bufs=1))
        kvw_p = ctx.enter_context(tc.tile_pool(name="kvw", bufs=1))
        tx_p = ctx.enter_context(tc.tile_pool(name="txp", bufs=1))
        x_p = ctx.enter_context(tc.tile_pool(name="x", bufs=1))
        q_p = ctx.enter_context(tc.tile_pool(name="q", bufs=2))
        k_p = ctx.enter_context(tc.tile_pool(name="k", bufs=1))
        vt_p = ctx.enter_context(tc.tile_pool(name="vt", bufs=1))
        e_p = ctx.enter_context(tc.tile_pool(name="e", bufs=4))
        ri_p = ctx.enter_context(tc.tile_pool(name="ri", bufs=3))
        on_p = ctx.enter_context(tc.tile_pool(name="on", bufs=1))
        os_p = ctx.enter_context(tc.tile_pool(name="os", bufs=4))
        ps = ctx.enter_context(tc.tile_pool(name="ps", bufs=8, space="PSUM"))

        # ---- DMA configs ----------------------------------------------
        # DMA engines are shared round-robin across active queues, so all
        # inputs go on ONE queue (sync) in exact first-need order - the
        # critical prologue transfers (tx+wkvk for k-proj) then get the
        # full aggregate DMA bandwidth instead of 1/3 of it.
        H = KC * C // 2
        Q4 = KC * C // 4
        XH = KC * NCHUNK // 2
        TXH = KC * TP // 2
        txs = tx_p.tile([128, KC * TP], bf16, name="txs", tag="tx")
        nc.sync.dma_start(txs[:, 0:TXH], tx[:, 0:TXH])
        nc.sync.dma_start(txs[:, TXH:], tx[:, TXH:])
        wkvk_sb = kvw_p.tile([128, KC * C], bf16, name="wkvk_sb", tag="wk")
        for j in range(KC):
            nc.sync.dma_start(wkvk_sb[:, j * C:(j + 1) * C],
                              wkvk[:, j * C:(j + 1) * C])
        cf_sb = c_p.tile([128, 3 * KC], f32, name="cf_sb", tag="cf")
        nc.sync.dma_start(cf_sb, cf)
        ones_sb = c_p.tile([128, TP], bf16, name="ones_sb", tag="o1")
        nc.sync.dma_start(ones_sb, onesd)
        qdt = fp8 if Q_FP8 else bf16
        xs = [x_p.tile([128, KC * NCHUNK], qdt, name=f"x{i}", tag=f"x{i}")
              for i in range(NPAIR)]
        wq_sb = w_p.tile([128, KC * C], qdt, name="wq_sb", tag="wq")
        XQ = KC * NCHUNK // 4
        for j in range(4):
            nc.sync.dma_start(xs[0][:, j * XQ:(j + 1) * XQ],
                              xd[0, :, j * XQ:(j + 1) * XQ])
            nc.sync.dma_start(wq_sb[:, j * Q4:(j + 1) * Q4],
                              wq[:, j * Q4:(j + 1) * Q4])
        if Q_FP8:
            # DoubleRow views: [p, group, plane(2), inner]
            xv = [xt[:, :].rearrange("p (g two n) -> p g two n",
                                     g=NG, two=2) for xt in xs]
            wqv = wq_sb[:, :].rearrange("p (g two d) -> p g two d",
                                        g=NG, two=2)
        wkvv_sb = kvw_p.tile([128, KC * C], bf16, name="wkvv_sb", tag="wv")
        nc.sync.dma_start(wkvv_sb[:, 0:H], wkvv[:, 0:H])
        nc.sync.dma_start(wkvv_sb[:, H:], wkvv[:, H:])
        bv_sb = c_p.tile([1, C], bf16, name="bv_sb", tag="bv")
        nc.sync.dma_start(bv_sb, bvr)
        nc.sync.dma_start(xs[1][:, 0:XH], xd[1, :, 0:XH])
        nc.sync.dma_start(xs[1][:, XH:], xd[1, :, XH:])
        wp_sb = w_p.tile([128, KC * C], bf16, name="wp_sb", tag="wp")
        nc.sync.dma_start(wp_sb[:, 0:H], wp[:, 0:H])
        nc.sync.dma_start(wp_sb[:, H:], wp[:, H:])
        for i in (2, 3):
            nc.sync.dma_start(xs[i][:, 0:XH], xd[i, :, 0:XH])
            nc.sync.dma_start(xs[i][:, XH:], xd[i, :, XH:])

        bq_c = cf_sb[:, 0:KC]
        bp_c = cf_sb[:, KC:2 * KC]
        bk_c = cf_sb[:, 2 * KC:3 * KC]

        # ---- k-proj: both batches packed along T ----------------------
        kps = [ps.tile([128, TP], f32, name=f"kps{h}", tag="ps")
               for h in range(NH)]
        for cc in range(KC):
            rhs = txs[:, cc * TP:(cc + 1) * TP]
            for h in range(NH):
                lhs = wkvk_sb[:, cc * C + h * HD:cc * C + (h + 1) * HD]
                nc.tensor.matmul(kps[h], lhs, rhs,
                                 start=(cc == 0), stop=(cc == KC - 1))
        k_sb = k_p.tile([128, NH * TP], bf16, name="k_sb", tag="k")
        for h in range(NH):
            nc.scalar.activation(k_sb[:, h * TP:(h + 1) * TP], kps[h], Id,
                                 bias=bk_c[:, h:h + 1])

        # ---- q-proj (fp8 DoubleRow when Q_FP8, else bf16) -------------
        DR = mybir.MatmulPerfMode.DoubleRow
        QDS = 1.0 / (QSW * QSX) if Q_FP8 else 1.0

        def qp_mms(qps, i, d, g0, g1):
            """Contraction-group matmuls g0..g1-1 for d-tile d of pair i."""
            if Q_FP8:
                for g in range(g0, g1):
                    nc.tensor.matmul(qps, wqv[:, g, :, d * HD:(d + 1) * HD],
                                     xv[i][:, g], start=(g == 0),
                                     stop=(g == NG - 1), perf_mode=DR)
            else:
                for cc in range(2 * g0, 2 * g1):
                    lhs = wq_sb[:, cc * C + d * HD:cc * C + (d + 1) * HD]
                    nc.tensor.matmul(qps, lhs,
                                     xs[i][:, cc * NCHUNK:(cc + 1) * NCHUNK],
                                     start=(cc == 0), stop=(cc == KC - 1))

        def qproj(i, interleave=None):
            """Emit q-proj for pair i. interleave=None -> group-outer (8
            psums, paced by DMA); else returns 8 per-d-tile emit closures."""
            q_sb = q_p.tile([128, KC * NCHUNK], bf16, name=f"q{i}", tag="q")

            def evac(d, qps):
                nc.scalar.activation(
                    q_sb[:, d * NCHUNK:(d + 1) * NCHUNK], qps, Id,
                    scale=QDS, bias=bq_c[:, d:d + 1])

            def emit_d(d):
                qps = ps.tile([128, NCHUNK], f32, name=f"qps{i}_{d}",
                              tag="ps")
                qp_mms(qps, i, d, 0, NG)
                evac(d, qps)

            if interleave is None:
                qps_l = [ps.tile([128, NCHUNK], f32, name=f"qps{i}_{d}",
                                 tag="ps") for d in range(KC)]
                for g in range(NG):
                    for d in range(KC):
                        qp_mms(qps_l[d], i, d, g, g + 1)
                for d in range(KC):
                    evac(d, qps_l[d])
                return q_sb
            return q_sb, [lambda d=d: emit_d(d) for d in range(KC)]

        q_t = [None] * NPAIR
        q_t[0] = qproj(0)

        # ---- v-proj: vt[b] = (Tx[b]^T Wv^T + bv) as [77, C] -----------
        vt_sb = []
        for b in range(BPC):
            vt = vt_p.tile([T, C], bf16, name=f"vt{b}", tag=f"vt{b}")
            for half in range(2):
                vps = ps.tile([T, NCHUNK], f32, name=f"vps{b}_{half}",
                              tag="ps")
                for cc in range(KC):
                    lhsT = txs[:, cc * TP + b * TB:cc * TP + b * TB + T]
                    rhs = wkvv_sb[:, cc * C + half * NCHUNK:
                                  cc * C + (half + 1) * NCHUNK]
                    nc.tensor.matmul(vps, lhsT, rhs,
                                     start=(cc == 0), stop=False)
                nc.tensor.matmul(vps, ones_sb[0:1, 0:T],
                                 bv_sb[:, half * NCHUNK:(half + 1) * NCHUNK],
                                 start=False, stop=True)
                nc.scalar.copy(vt[:, half * NCHUNK:(half + 1) * NCHUNK], vps)
            vt_sb.append(vt)

        # ---- chunk loop: A(i) interleaved with partner stream ---------
        on_t = [[None] * NH for _ in range(NPAIR)]

        def attention(i, partner):
            """Emit attention for pair i, interleaving partner closures."""
            b = i // NCH
            q_sb = q_t[i]
            sps_l = {}
            e_l = {}

            def sc(h):
                sps = ps.tile([T, NCHUNK], f32, name=f"sps{i}_{h}", tag="ps")
                nc.tensor.matmul(sps, k_sb[:, h * TP + b * TB:
                                           h * TP + b * TB + T],
                                 q_sb[:, h * NCHUNK:(h + 1) * NCHUNK])
                e_sb = e_p.tile([T, NCHUNK], bf16, name=f"e{i}_{h}", tag="e")
                nc.scalar.activation(e_sb, sps, Exp, scale=SCALE)
                e_l[h] = e_sb

            def dn_out(h):
                rps = ps.tile([128, NCHUNK], f32, name=f"rps{i}_{h}",
                              tag="ps")
                nc.tensor.matmul(rps, ones_sb[0:T, 0:128], e_l[h])
                ri = ri_p.tile([128, NCHUNK], f32, name=f"ri{i}_{h}",
                               tag="ri")
                nc.vector.reciprocal_approx_fast(ri, rps)
                ops = ps.tile([128, NCHUNK], f32, name=f"ops{i}_{h}",
                              tag="ps")
                nc.tensor.matmul(ops, vt_sb[b][:, h * HD:(h + 1) * HD],
                                 e_l[h])
                on = on_p.tile([128, NCHUNK], bf16, name=f"on{i}_{h}",
                               tag=f"on{i}_{h}")
                nc.vector.tensor_mul(on, ops, ri)
                on_t[i][h] = on

            p = list(partner)
            sc(0)
            sc(1)
            if p:
                p.pop(0)()
            sc(2)
            for h in range(NH):
                dn_out(h)
                if h + 3 < NH:
                    sc(h + 3)
                if p:
                    p.pop(0)()
            while p:
                p.pop(0)()

        def pproj(i, interleave=False):
            """Emit p-proj for pair i; returns 8 closures if interleave."""
            def emit_e(e):
                fps = ps.tile([128, NCHUNK], f32, name=f"fps{i}_{e}",
                              tag="ps")
                for d in range(KC):
                    lhs = wp_sb[:, d * C + e * HD:d * C + (e + 1) * HD]
                    nc.tensor.matmul(fps, lhs, on_t[i][d],
                                     start=(d == 0), stop=(d == KC - 1))
                osb = os_p.tile([128, NCHUNK], bf16, name=f"os{i}_{e}",
                                tag="os")
                hm = NCHUNK // 2
                o0 = e * NCHUNK
                nc.scalar.activation(osb[:, 0:hm], fps[:, 0:hm], Id,
                                     bias=bp_c[:, e:e + 1])
                nc.sync.dma_start(out[i, :, o0:o0 + hm], osb[:, 0:hm])
                nc.scalar.activation(osb[:, hm:], fps[:, hm:], Id,
                                     bias=bp_c[:, e:e + 1])
                nc.sync.dma_start(out[i, :, o0 + hm:o0 + NCHUNK],
                                  osb[:, hm:])

            cl = [lambda e=e: emit_e(e) for e in range(KC)]
            if interleave:
                return cl
            for c in cl:
                c()

        # A0+qp1, A1+qp2, A2+qp3, A3+pp0, pp1, pp2, pp3
        for i in range(NPAIR):
            if i + 1 < NPAIR:
                q_t[i + 1], partner = qproj(i + 1, interleave=True)
            else:
                partner = pproj(0, interleave=True)
            attention(i, partner)
        for i in range(1, NPAIR):
            pproj(i)

    nc.compile()
    return nc


def _host_prep(Vx, Tx, Wq, bq, Wkv, bkv, Wp, bp):
    import ml_dtypes
    bf = ml_dtypes.bfloat16
    f = np.float32

    def wtile(w_t):
        # [C(row c), C(col d)] -> [128, KC*C] with block cc at cols cc*C
        return np.ascontiguousarray(
            w_t.reshape(KC, 128, C).transpose(1, 0, 2).reshape(128, KC * C)
        ).astype(bf)

    def q8(v, s):
        return np.clip(np.asarray(v, f) * s, -240.0,
                       240.0).astype(ml_dtypes.float8_e4m3)

    Wq = np.asarray(Wq, f)
    Wkv4 = np.asarray(Wkv, f).reshape(NH, 2, HD, C)
    Wp = np.asarray(Wp, f)
    if Q_FP8:
        # DoubleRow layout: flat col = g*2*C + plane*C + d,
        # value = Wq.T[g*256 + plane*128 + p, d] * QSW in fp8e4
        wq_h = np.ascontiguousarray(
            q8(Wq.T, QSW).reshape(NG, 2, 128, C)
            .transpose(2, 0, 1, 3).reshape(128, KC * C))
    else:
        wq_h = wtile(Wq.T)
    wkvk_h = wtile(Wkv4[:, 0].reshape(C, C).T)
    wkvv_h = wtile(Wkv4[:, 1].reshape(C, C).T)
    wp_h = wtile(Wp.T)

    cf32 = np.zeros((128, 3 * KC), f)
    cf32[:, 0:KC] = np.asarray(bq, f).reshape(KC, 128).T
    cf32[:, KC:2 * KC] = np.asarray(bp, f).reshape(KC, 128).T
    bkv3 = np.asarray(bkv, f).reshape(NH, 2, HD)
    cf32[:, 2 * KC:3 * KC] = bkv3[:, 0].T
    bvr_h = np.ascontiguousarray(bkv3[:, 1].reshape(1, C)).astype(bf)
    ones_h = np.ones((128, TP), bf)

    Vx3 = np.asarray(Vx, f).reshape(B, C, N)
    TxA = np.asarray(Tx, f)

    shared = {"wq": wq_h, "wkvk": wkvk_h, "wkvv": wkvv_h, "wp": wp_h,
              "cf32": cf32, "onesd": ones_h, "bvr": bvr_h}
    in_maps = []
    for i in range(NCORES):
        m = dict(shared)
        xb = Vx3[i * BPC:(i + 1) * BPC]
        if Q_FP8:
            m["x"] = np.ascontiguousarray(
                q8(xb, QSX).reshape(BPC, NG, 2, 128, NCH, NCHUNK)
                .transpose(0, 4, 3, 1, 2, 5)
                .reshape(NPAIR, 128, KC * NCHUNK))
        else:
            m["x"] = np.ascontiguousarray(
                xb.reshape(BPC, KC, 128, NCH, NCHUNK)
                .transpose(0, 3, 2, 1, 4).reshape(NPAIR, 128, KC * NCHUNK)
            ).astype(bf)
        txh = np.zeros((128, KC * TP), f)
        for cc in range(KC):
            for b2 in range(BPC):
                txh[:, cc * TP + b2 * TB:cc * TP + b2 * TB + T] = \
                    TxA[i * BPC + b2, cc * 128:(cc + 1) * 128, :]
        m["tx"] = txh.astype(bf)
        in_maps.append(m)
    return in_maps


def _unshard_core(arr):
    """[NPAIR, 128, KC*NCHUNK] bf16 -> [BPC, C, N] float32."""
    a = np.asarray(arr).astype(np.float32)
    return (a.reshape(BPC, NCH, 128, KC, NCHUNK)
            .transpose(0, 3, 2, 1, 4).reshape(BPC, C, N))


def get_module():
    if "nc" not in _CACHE:
        _CACHE["nc"] = _build_module()
    return _CACHE["nc"]


def kernel(**inputs):
    from concourse.bass_utils import run_bass_kernel_spmd

    nc = get_module()
    in_maps = _host_prep(**inputs)
    res = run_bass_kernel_spmd(nc, in_maps, core_ids=list(range(NCORES)))
    outs = [_unshard_core(res.results[i]["out"]) for i in range(NCORES)]
    full = np.concatenate(outs, axis=0).reshape(B, C, 32, 32)
    return np.ascontiguousarray(full.astype(np.float32))
